# revision 1
# baseline (speedup 1.0000x reference)
"""DepthAwareBokehDFN Trainium2 kernel, v2.

Per image: x = concat(rgb, depth) (4ch) -> conv3x3(64)+relu ->
conv3x3(64)+relu -> conv3x3(81) -> softmax over 81 taps -> 9x9 dynamic
filtering of rgb.  Data parallel over 8 cores; shard = (batch, H-half),
R=192 output rows per core.  Halos recomputed from DRAM.

v2 dataflow per core (row-streamed convs, strip-wise bokeh):
  - convs as shift-matmuls in fp16, channels on partitions, 1-row tiles
    N=384.  K=128 (kh,kh+1) pairing via windows whose partitions 64:128
    hold the NEXT row; window fills are SBUF-SBUF DMAs.
  - conv1/conv2 row pairs share one PSUM tile (even row at partitions
    0:64, odd at 64:128 = disjoint PE column groups, concurrent chains)
    evacuated by ONE ACT op per pair.
  - conv3+exp: ACT writes exp(logits+b3) into ESTG2[t, l*128+p] where
    p = 4*r' + qx (r' = row in 32-row strip, qx = quarter of the row,
    l = pixel in quarter).  One XBAR dma_start_transpose per strip
    turns that into eb[p, l*96+t] (tap-minor).
  - bokeh per strip: partition p holds its own rgb halo block
    (prescattered on host, 3ch x 9dy x 104w fp16), tap products via one
    DVE mul per channel iterating (l, dy, dx), tap sums via DVE
    tensor_reduce over the contiguous tap axis (two halves in fp16,
    combined in fp32), normalize with reciprocal.
"""

import os
import sys
import numpy as np

if "/opt/trn_rl_repo" not in sys.path:
    sys.path.insert(0, "/opt/trn_rl_repo")

import concourse.bass as bass  # noqa: E402
import concourse.bacc as bacc  # noqa: E402
import concourse.mybir as mybir  # noqa: E402
import concourse.tile as tile  # noqa: E402

F32 = mybir.dt.float32
F16 = mybir.dt.float16

B, H, W = 4, 384, 384
NC_ = 8          # cores
RS = 392         # row slot stride (elements) in window / x36 buffers
QW = 96          # quarter-row width
SR = 32          # rows per bokeh strip (=> 128 partitions = 4*32)
NSLOT = 8        # rows kept in h1/h2 windows
NSLOTX = 4       # row-PAIR slots kept in x36 window
RGBW = 104       # rgb halo block row width (96 + 8)
RGBF = 27 * RGBW  # rgb halo block elems per partition (3ch * 9dy * 104)
EB = 96 * 96     # eb free elems per partition (l-major, 96 tap slots)
ESF = SR * W     # ESTG2 free elems (12288)

# weight-table column layout (fp16 table)
C_L1 = 0                  # block-diag conv1 lhsT (72,128) at rows 0:72
C_L2A = 128               # 3x (128,64)
C_L2B = C_L2A + 192       # 3x (64,64)  at rows 64:128
C_L3A = C_L2B + 192       # 3x (128,81)
C_L3B = C_L3A + 243       # 3x (64,81)  at rows 64:128
WCOLS = C_L3B + 243

AF = mybir.ActivationFunctionType
ALU = mybir.AluOpType


def build_core_program(R=192, dbg_tap=None):
    """Per-core Bass program.  R = output rows (multiple of SR)."""
    assert R % SR == 0
    nstrip = R // SR

    nc = bacc.Bacc("TRN2", debug=False, enable_asserts=False,
                   num_devices=NC_, enable_partition_id=False,
                   num_swdge_queues=4)

    x36d = nc.dram_tensor("x36d", [72, (R + 4) // 2, RS], F16,
                          kind="ExternalInput").ap()
    rgbs = nc.dram_tensor("rgbs", [nstrip * 128, RGBF], F16,
                          kind="ExternalInput").ap()
    wtsb = nc.dram_tensor("wtsb", [128, WCOLS], F16,
                          kind="ExternalInput").ap()
    wtb = nc.dram_tensor("wtb", [128, 3], F32, kind="ExternalInput").ap()
    out = nc.dram_tensor("out", [3, R, W], F32, kind="ExternalOutput").ap()
    dbg = None
    if dbg_tap == "E":
        dbg = nc.dram_tensor("dbg", [nstrip, 96, ESF], F16,
                             kind="ExternalOutput").ap()
    elif dbg_tap == "eb":
        dbg = nc.dram_tensor("dbg", [nstrip, 128, EB], F16,
                             kind="ExternalOutput").ap()
    elif dbg_tap == "h1":
        dbg = nc.dram_tensor("dbg", [64, R + 4, RS], F16,
                             kind="ExternalOutput").ap()
    elif dbg_tap == "h2":
        dbg = nc.dram_tensor("dbg", [64, R + 2, RS], F16,
                             kind="ExternalOutput").ap()

    def s1(j):  # h1 window slot of row j (rows -2 .. R+1)
        return (j + 2) % NSLOT

    def s2(j):  # h2 window slot (rows -1 .. R)
        return (j + 1) % NSLOT

    def sx(j):  # x36 pair-slot of conv1-out pair starting at row j
        return ((j + 2) // 2) % NSLOTX

    with tile.TileContext(nc) as tc:
        with (
            tc.tile_pool(name="singles", bufs=1) as singles,
            tc.tile_pool(name="ostg_pool", bufs=2) as ostg_pool,
            tc.tile_pool(name="estg_pool", bufs=4) as estg_pool,
            tc.tile_pool(name="psum", bufs=2, space="PSUM") as psum,
            tc.tile_pool(name="psumB", bufs=1, space="PSUM") as psumB,
        ):
            # ---- persistent SBUF state ----
            wtsb_sb = singles.tile([128, WCOLS], F16)
            nc.sync.dma_start(out=wtsb_sb, in_=wtsb)
            wtb_sb = singles.tile([128, 3], F32)
            nc.sync.dma_start(out=wtb_sb, in_=wtb)
            h1w = singles.tile([128, NSLOT * RS], F16)
            h2w = singles.tile([128, NSLOT * RS], F16)
            x36 = singles.tile([72, NSLOTX * RS], F16)
            estg = [singles.tile([96, ESF], F16, name=f"estg{i}")
                    for i in range(2)]
            eb = singles.tile([128, EB], F16)
            rgbb = [singles.tile([128, RGBF], F16, name=f"rgbb{i}")
                    for i in range(2)]
            tmpP = singles.tile([128, 96 * 81], F16)
            scrA = singles.tile([128, 70 * QW], F16)
            uh = singles.tile([128, 2, QW], F16)
            uacc = singles.tile([128, 4, QW], F32)

            nc.vector.memset(h1w, 0.0)
            nc.vector.memset(h2w, 0.0)
            nc.vector.memset(x36, 0.0)
            for es_ in estg:
                nc.vector.memset(es_[0:96, :], 0.0)

            # weight slices
            l1 = wtsb_sb[0:72, C_L1:C_L1 + 128]
            l2a = [wtsb_sb[0:128, C_L2A + 64 * k:C_L2A + 64 * (k + 1)]
                   for k in range(3)]
            l2b = [wtsb_sb[64:128, C_L2B + 64 * k:C_L2B + 64 * (k + 1)]
                   for k in range(3)]
            l3a = [wtsb_sb[0:128, C_L3A + 81 * k:C_L3A + 81 * (k + 1)]
                   for k in range(3)]
            l3b = [wtsb_sb[64:128, C_L3B + 81 * k:C_L3B + 81 * (k + 1)]
                   for k in range(3)]

            def bias(col, lo, hi):
                return wtb_sb[lo:hi, col:col + 1]

            # ---------------- emission helpers ----------------
            def emit_x36_batch(y0):
                # load pair-slots for pairs (y0, y0+2, ...), up to NSLOTX
                j2 = (y0 + 2) // 2
                n = min(NSLOTX, (R + 4) // 2 - j2)
                F = NSLOTX * RS
                dst = bass.AP(tensor=x36.tensor, offset=sx(y0) * RS,
                              ap=[[F, 72], [RS, n], [1, RS]])
                src = bass.AP(tensor=x36d.tensor, offset=j2 * RS,
                              ap=[[(R + 4) // 2 * RS, 72], [RS, n],
                                  [1, RS]])
                nc.gpsimd.dma_start(out=dst, in_=src)

            def emit_conv1(y, ps):
                # one K=72 block-diagonal matmul computes the full pair:
                # psum 0:64 = row y, 64:128 = row y+1
                rhs = x36[0:72, sx(y) * RS + 1: sx(y) * RS + 385]
                nc.tensor.matmul(out=ps[0:128, 0:384], lhsT=l1, rhs=rhs,
                                 start=True, stop=True)

            def emit_conv23_pair(rows, outs, win, sl, l_a, l_b):
                """Chunk-interleaved matmuls for a pair of conv rows
                sharing lhsT (halves weight swaps on the PE): rows =
                [(w, ps_ap) or None, ...]."""
                def rhs_of(w, ci):
                    if ci < 3:
                        return win[0:128, sl(w - 1) * RS + ci:
                                   sl(w - 1) * RS + ci + 384]
                    kw = ci - 3
                    return win[64:128, sl(w) * RS + kw:
                               sl(w) * RS + kw + 384]

                for ci in range(6):
                    lh = l_a[ci] if ci < 3 else l_b[ci - 3]
                    for ri, w in enumerate(rows):
                        if w is None:
                            continue
                        nc.tensor.matmul(out=outs[ri], lhsT=lh,
                                         rhs=rhs_of(w, ci),
                                         start=(ci == 0), stop=(ci == 5))

            def emit_evac12(ps, win, slot_idx, bias_ap):
                # one ACT op: relu(psum + bias) for the full row pair
                nc.scalar.activation(
                    out=win[0:128, slot_idx * RS + 1: slot_idx * RS + 385],
                    in_=ps[0:128, 0:384], func=AF.Relu, bias=bias_ap)

            def emit_fill_c1(win, slot_of, j, eng):  # c1[j] <- c2[j-1]
                a, b_ = slot_of(j), slot_of(j - 1)
                eng.dma_start(
                    out=win[0:64, a * RS:(a + 1) * RS],
                    in_=win[64:128, b_ * RS:(b_ + 1) * RS])

            def emit_fill_c2(win, slot_of, j, eng):  # c2[j] <- c1[j+1]
                a, b_ = slot_of(j), slot_of(j + 1)
                eng.dma_start(
                    out=win[64:128, a * RS:(a + 1) * RS],
                    in_=win[0:64, b_ * RS:(b_ + 1) * RS])

            def emit_exp(v0, er, i, ps):
                # exp(logits+b3) of row v0+i into the pair staging tile
                nc.scalar.activation(out=er[0:81, 384 * i:384 * (i + 1)],
                                     in_=ps[0:81, 0:384],
                                     func=AF.Exp, bias=bias(2, 0, 81))

            def emit_scatter(v0, er):
                # scatter rows (v0, v0+1) into ESTG2[t, l*128+4*r'+qx];
                # (i, l, qx) iteration keeps dst runs 4-elem contiguous
                es = estg[(v0 // SR) % 2]
                r_ = v0 % SR
                dst = bass.AP(tensor=es.tensor, offset=4 * r_,
                              ap=[[ESF, 81], [4, 2], [128, 96], [1, 4]])
                src = bass.AP(tensor=er.tensor, offset=0,
                              ap=[[768, 81], [384, 2], [1, 96], [96, 4]])
                nc.vector.tensor_copy(out=dst, in_=src)

            def emit_rgb_dma(s):
                nc.gpsimd.dma_start(
                    out=rgbb[s % 2][0:128, :],
                    in_=rgbs[s * 128:(s + 1) * 128, :])

            def emit_xbar_q(s, q):
                # quarter-strip transpose (l in [24q, 24q+24)) so window
                # fills queued behind it on SP wait <=3us, not 12us
                es = estg[s % 2]
                dst = bass.AP(tensor=eb.tensor, offset=q * 24 * 96,
                              ap=[[EB, 128], [96, 24], [1, 96]])
                nc.sync.dma_start_transpose(
                    out=dst, in_=es[0:96, q * 3072:(q + 1) * 3072])
                if dbg_tap == "E" and q == 0:
                    nc.gpsimd.dma_start(out=dbg[s], in_=es[0:96, :])

            def emit_bokeh(s):
                if dbg_tap == "eb":
                    nc.gpsimd.dma_start(out=dbg[s], in_=eb[0:128, :])
                ostg = ostg_pool.tile([128, 3, QW], F32, name=f"ostg{s}",
                                      tag="ostg")
                with nc.allow_low_precision("fp16 bokeh by design"):
                    for ch in range(4):
                        if ch < 3:
                            # tmpP[(l,dy,dx)] = E * rgb_shift  (tap-minor)
                            dst = bass.AP(
                                tensor=tmpP.tensor, offset=0,
                                ap=[[96 * 81, 128], [81, 96], [9, 9],
                                    [1, 9]])
                            ein = bass.AP(
                                tensor=eb.tensor, offset=0,
                                ap=[[EB, 128], [96, 96], [9, 9], [1, 9]])
                            rin = bass.AP(
                                tensor=rgbb[s % 2].tensor,
                                offset=ch * 9 * RGBW,
                                ap=[[RGBF, 128], [1, 96], [RGBW, 9],
                                    [1, 9]])
                            nc.vector.tensor_mul(dst, ein, rin)
                            src_t, tst = tmpP, 81
                        else:
                            src_t, tst = eb, 96
                        # tap-sum: fp16 fold chain 80->40->20->10 (2x DVE
                        # mode), short reduce, then +plane80 in fp32
                        srcf = 96 * tst

                        def sap(off, cnt, t0=0):
                            return bass.AP(tensor=src_t.tensor,
                                           offset=t0,
                                           ap=[[srcf, 128], [tst, 96],
                                               [1, cnt]])

                        def fap(base, width, cnt, t0=0):
                            return bass.AP(tensor=scrA.tensor,
                                           offset=base * QW + t0,
                                           ap=[[70 * QW, 128],
                                               [width, 96], [1, cnt]])

                        nc.vector.tensor_add(fap(0, 40, 40),
                                             sap(0, 40),
                                             sap(0, 40, t0=40))
                        nc.vector.tensor_add(fap(40, 20, 20),
                                             fap(0, 40, 20),
                                             fap(0, 40, 20, t0=20))
                        nc.vector.tensor_add(fap(60, 10, 10),
                                             fap(40, 20, 10),
                                             fap(40, 20, 10, t0=10))
                        nc.vector.tensor_reduce(
                            out=uh[0:128, 0, :],
                            in_=fap(60, 10, 10),
                            axis=mybir.AxisListType.X, op=ALU.add)
                        nc.vector.tensor_add(
                            uacc[0:128, ch, :], uh[0:128, 0, :],
                            bass.AP(tensor=src_t.tensor, offset=80,
                                    ap=[[srcf, 128], [tst, 96]]))

                    nc.vector.reciprocal(uacc[0:128, 3, :],
                                         uacc[0:128, 3, :])
                    for ch in range(3):
                        nc.vector.tensor_mul(ostg[0:128, ch, :],
                                             uacc[0:128, ch, :],
                                             uacc[0:128, 3, :])

                for ch in range(3):
                    dst = bass.AP(tensor=out.tensor,
                                  offset=ch * R * W + s * SR * W,
                                  ap=[[W, SR], [QW, 4], [1, QW]])
                    src = bass.AP(tensor=ostg.tensor, offset=ch * QW,
                                  ap=[[3 * QW, 128], [1, QW]])
                    nc.gpsimd.dma_start(out=dst, in_=src)

            # ---------------- main row loop ----------------
            emit_rgb_dma(0)
            for k in range((R + 8) // 2):
                y = -2 + 2 * k          # conv1 pair (y, y+1)
                if y <= R:
                    if ((y + 2) // 2) % NSLOTX == 0:
                        emit_x36_batch(y)
                    ps1 = psumB.tile([128, 384], F32, tag="c1",
                                    name=f"c1_{k}")
                    emit_conv1(y, ps1)
                    emit_evac12(ps1, h1w, s1(y), bias(0, 0, 128))
                    emit_fill_c1(h1w, s1, y + 1, nc.scalar)
                    if y >= 0:
                        emit_fill_c2(h1w, s1, y - 1, nc.scalar)
                    if dbg_tap == "h1":
                        nc.gpsimd.dma_start(
                            out=dbg[:, y + 2, :],
                            in_=h1w[0:64, s1(y) * RS:(s1(y) + 1) * RS])
                        nc.gpsimd.dma_start(
                            out=dbg[:, y + 3, :],
                            in_=h1w[64:128, s1(y) * RS:(s1(y) + 1) * RS])

                # conv2 pair (y-2, y-1); computed conv2 rows are 0..R
                # (h2 row -1 stays zero: image-edge padding, see host flip)
                w0 = y - 2
                wrote = [0 <= w0 <= R, 0 <= w0 + 1 <= R]
                if any(wrote):
                    # even/odd rows in separate PSUM banks so the
                    # chunk-interleaved accumulation groups are legal
                    pse = psum.tile([128, 384], F32, tag="c2e",
                                     name=f"c2e_{k}") if wrote[0] else None
                    pso = psum.tile([128, 384], F32, tag="c2o",
                                     name=f"c2o_{k}") if wrote[1] else None
                    emit_conv23_pair(
                        [w0 if wrote[0] else None,
                         w0 + 1 if wrote[1] else None],
                        [pse[0:64, 0:384] if pse is not None else None,
                         pso[64:128, 0:384] if pso is not None else None],
                        h1w, s1, l2a, l2b)
                    sl_ = s2(w0)
                    if wrote[0]:
                        nc.scalar.activation(
                            out=h2w[0:64, sl_ * RS + 1: sl_ * RS + 385],
                            in_=pse[0:64, 0:384], func=AF.Relu,
                            bias=bias(1, 0, 64))
                    if wrote[1]:
                        nc.scalar.activation(
                            out=h2w[64:128, sl_ * RS + 1: sl_ * RS + 385],
                            in_=pso[64:128, 0:384], func=AF.Relu,
                            bias=bias(1, 64, 128))
                    if wrote[1]:
                        emit_fill_c1(h2w, s2, w0 + 1, nc.sync)
                    if wrote[0] and w0 >= 0:
                        emit_fill_c2(h2w, s2, w0 - 1, nc.sync)
                    if dbg_tap == "h2":
                        sl_ = s2(w0)
                        nc.gpsimd.dma_start(
                            out=dbg[:, w0 + 1, :],
                            in_=h2w[0:64, sl_ * RS:(sl_ + 1) * RS])
                        nc.gpsimd.dma_start(
                            out=dbg[:, w0 + 2, :],
                            in_=h2w[64:128, sl_ * RS:(sl_ + 1) * RS])

                # conv3 pair (y-4, y-3); valid rows 0..R-1
                v0 = y - 4
                vrows = [v0 if 0 <= v0 < R else None,
                         v0 + 1 if 0 <= v0 + 1 < R else None]
                if any(v is not None for v in vrows):
                    pss = [psumB.tile([128, 384], F32, tag=t,
                                     name=f"c3_{v0}_{t}")
                           if v is not None else None
                           for v, t in zip(vrows, ("c3e", "c3o"))]
                    emit_conv23_pair(
                        vrows,
                        [p[0:81, 0:384] if p is not None else None
                         for p in pss],
                        h2w, s2, l3a, l3b)
                    er = estg_pool.tile([81, 768], F16, name=f"er_{v0}",
                                        tag="er")
                    for i, (v, ps3) in enumerate(zip(vrows, pss)):
                        if v is not None:
                            emit_exp(v0, er, i, ps3)
                    emit_scatter(v0, er)
                    if (v0 + 1) % SR == SR - 1:
                        s = v0 // SR
                        for q in range(4):
                            emit_xbar_q(s, q)
                        if s + 1 < nstrip:
                            emit_rgb_dma(s + 1)
                        emit_bokeh(s)

    nc.compile()
    return nc


# ------------------------- host side -------------------------

def prep_weights(w1, b1, w2, b2, w3, b3, flip=False):
    if flip:
        perm = np.array([(8 - t // 9) * 9 + t % 9 for t in range(81)])
        w1 = w1[:, :, ::-1, :]
        w2 = w2[:, :, ::-1, :]
        w3 = w3[perm][:, :, ::-1, :]
        b3 = b3[perm]
    wtsb = np.zeros((128, WCOLS), np.float32)
    l1 = w1.transpose(2, 3, 1, 0).reshape(36, 64)
    wtsb[0:36, C_L1:C_L1 + 64] = l1
    wtsb[36:72, C_L1 + 64:C_L1 + 128] = l1
    for k in range(3):
        wtsb[0:64, C_L2A + 64 * k:C_L2A + 64 * (k + 1)] = w2[:, :, 0, k].T
        wtsb[64:128, C_L2A + 64 * k:C_L2A + 64 * (k + 1)] = w2[:, :, 1, k].T
        wtsb[64:128, C_L2B + 64 * k:C_L2B + 64 * (k + 1)] = w2[:, :, 2, k].T
        wtsb[0:64, C_L3A + 81 * k:C_L3A + 81 * (k + 1)] = w3[:, :, 0, k].T
        wtsb[64:128, C_L3A + 81 * k:C_L3A + 81 * (k + 1)] = w3[:, :, 1, k].T
        wtsb[64:128, C_L3B + 81 * k:C_L3B + 81 * (k + 1)] = w3[:, :, 2, k].T
    wtb = np.zeros((128, 3), np.float32)
    wtb[0:64, 0] = b1
    wtb[64:128, 0] = b1
    wtb[0:64, 1] = b2
    wtb[64:128, 1] = b2
    wtb[0:81, 2] = b3
    return wtsb.astype(np.float16), wtb


def prep_shard(x, rgb_b, R):
    """x: (4,H,W) fp32 of one (possibly flipped) image; rgb_b: (3,H,W).
    Shard = rows 0..R; top edge is the image edge (zero pad), bottom
    halo rows R..R+3 come from the rest of the image.

    Returns (x36d, rgbs): pair-packed im2col'd conv1 input and per-strip
    rgb halo blocks in the quarter-row layout."""
    # padded x rows -3 .. R+3, width 392 (img col x at 1+x)
    xp = np.zeros((4, R + 6, RS), np.float32)
    hi = min(R + 3, H)
    xp[:, 3:3 + hi, 1:385] = x[:, 0:hi, :]
    x36f = np.zeros((36, R + 4, RS), np.float16)
    for kh in range(3):
        for kw in range(3):
            blk = np.zeros((4, R + 4, RS), np.float32)
            if kw == 0:
                blk[:, :, 1:] = xp[:, kh:kh + R + 4, :-1]
            elif kw == 1:
                blk[:, :, :] = xp[:, kh:kh + R + 4, :]
            else:
                blk[:, :, :-1] = xp[:, kh:kh + R + 4, 1:]
            for c in range(4):
                x36f[kh * 12 + kw * 4 + c] = blk[c].astype(np.float16)
    # conv1-out row -1 must be exactly zero (image-edge h1 padding)
    x36f[:, 1, :] = 0
    # pair-pack: slot j2 = (y+2)//2; partitions 0:36 = row y, 36:72 = y+1
    x36d = np.zeros((72, (R + 4) // 2, RS), np.float16)
    x36d[0:36] = x36f[:, 0::2, :]
    x36d[36:72] = x36f[:, 1::2, :]

    # rgb halo rows -4 .. R+4, quarter-row blocks
    rgbp = np.zeros((3, R + 8, W + 8), np.float32)
    hi2 = min(R + 4, H)
    rgbp[:, 4:4 + hi2, 4:4 + W] = rgb_b[:, 0:hi2, :]
    nstrip = R // SR
    arr = np.zeros((nstrip * 128, RGBF), np.float16)
    # partition p = s*128 + 4*r' + qx; free = (c*9 + dy)*RGBW + w
    # value = rgbp[c, s*SR + r' + dy, qx*96 + w]
    rows = rgbp.astype(np.float16)  # (3, R+8, 392)
    for s in range(nstrip):
        for dy in range(9):
            seg = rows[:, s * SR + dy:s * SR + dy + SR, :]  # (3,SR,392)
            for qx in range(4):
                qseg = seg[:, :, qx * 96:qx * 96 + RGBW]  # (3,SR,104)
                view = arr[s * 128 + qx:s * 128 + 128 + qx:4]
                for c in range(3):
                    view[:, (c * 9 + dy) * RGBW:
                         (c * 9 + dy + 1) * RGBW] = qseg[c]
    return x36d, arr


def _prep_inputs(rgb, depth, w1, b1, w2, b2, w3, b3):
    R = H // 2
    x = np.concatenate([rgb, depth], axis=1)  # (B,4,H,W)
    wt_n = prep_weights(w1, b1, w2, b2, w3, b3, flip=False)
    wt_f = prep_weights(w1, b1, w2, b2, w3, b3, flip=True)
    in_maps = []
    for core in range(NC_):
        bi, half = divmod(core, 2)
        if half == 0:
            xi, ri = x[bi], rgb[bi]
            wtsb, wtb = wt_n
        else:
            xi, ri = x[bi, :, ::-1, :], rgb[bi, :, ::-1, :]
            wtsb, wtb = wt_f
        x36d, rgbs = prep_shard(xi, ri, R)
        in_maps.append({"x36d": x36d, "rgbs": rgbs,
                        "wtsb": wtsb, "wtb": wtb})
    return in_maps


_CACHE = {}


def _get_program(R=H // 2, dbg_tap=None):
    key = (R, dbg_tap)
    if key not in _CACHE:
        _CACHE[key] = build_core_program(R, dbg_tap)
    return _CACHE[key]


def kernel(rgb, depth, w1, b1, w2, b2, w3, b3):
    from concourse.bass_utils import run_bass_kernel_spmd
    rgb = np.asarray(rgb, np.float32)
    depth = np.asarray(depth, np.float32)
    nc = _get_program()
    in_maps = _prep_inputs(rgb, depth, np.asarray(w1, np.float32),
                           np.asarray(b1, np.float32),
                           np.asarray(w2, np.float32),
                           np.asarray(b2, np.float32),
                           np.asarray(w3, np.float32),
                           np.asarray(b3, np.float32))
    res = run_bass_kernel_spmd(nc, in_maps, core_ids=list(range(NC_)),
                               trace=bool(int(os.environ.get("KT_TRACE",
                                                             "0"))))
    R = H // 2
    outp = np.zeros((B, 3, H, W), np.float32)
    for core in range(NC_):
        bi, half = divmod(core, 2)
        o = res.results[core]["out"]
        if half == 0:
            outp[bi, :, 0:R, :] = o
        else:
            outp[bi, :, R:H, :] = o[:, ::-1, :]
    kernel.last_result = res
    return outp


if __name__ == "__main__":
    nc = build_core_program(R=int(sys.argv[1]) if len(sys.argv) > 1 else 32)
    print("built ok")

